# revision 12
# baseline (speedup 1.0000x reference)
"""Trainium2 Bass kernel for nn_DecoderBlock_36653250904785.

Decoder block: inverse-square-distance weighted interpolation of down-sampled
point features onto up-sampled points, concat with up features, then a 2-layer
pointwise MLP with eval-mode batchnorm (ReLU after layer 1).

Sharding: data-parallel over (batch, N/2) -> 8 cores, each core handles one
(b, half-of-N) slab: 8192 up-points x 2048 down-points.

Per-core device pipeline, in (m=down-point, n=up-point) orientation so that
the contraction dim of every matmul sits on SBUF partitions:
  1. dist^T tile [128m, 512n] via one K=25 bf16 matmul: squared distance
     expanded as an inner product of 3-way bf16 splits (error ~1e-6, at the
     fp32 reference's own noise floor), with +eps folded in as an extra row.
  2. r = 1/dist: elementwise reciprocal, split between the Vector engine
     (custom-DVE Newton iteration) and the Scalar engine (ACT spline
     Reciprocal), both writing float32r (PE-native 12-bit-mantissa fp32).
  3. feat_interp^T numerator and denominator in one PE accumulation:
     lhsT = [feat_down | 1] so PSUM row 64 is sum_m r.
  4. normalize: rs = 1/sum (DVE), broadcast across partitions (GpSimd),
     multiply (DVE) straight into the MLP input tile.
  5. MLP: two fp32r matmuls with folded-BN affine epilogues on ACT/DVE.
"""

import numpy as np
import ml_dtypes

import concourse.bass as bass
import concourse.mybir as mybir
import concourse.tile as tile
from concourse import bacc
from concourse.bass import ts
from concourse.bass_utils import run_bass_kernel_spmd
from concourse.dve_ops import RECIPROCAL_APPROX_FAST

P = 128
B, M, N, D, C = 4, 2048, 16384, 64, 64
NC = 8                      # cores
NN = N * B // NC            # 8192 n-rows per core
NT = 512                    # n-tile (one PSUM bank)
MT = M // P                 # 16 m-chunks
TT = NN // NT               # 16 n-tiles per core
KD = 25                     # dist matmul contraction rows
BN_EPS = 1e-5
DIST_EPS = 1e-8

dt = mybir.dt
AF = mybir.ActivationFunctionType
bf16 = ml_dtypes.bfloat16
RAF = dict(s0=-0.23549792, s1=2.0017324, imm2=2.0)

# of the 16 recip tiles per n-tile, how many go to the Scalar engine
N_ACT_RECIP = 9

_compiled_cache = {}


def _split3(x):
    """3-way bf16 split of fp64 array: x ~= h + m + l to ~2^-27 rel."""
    x = np.asarray(x, np.float64)
    h = x.astype(bf16)
    m = (x - h.astype(np.float64)).astype(bf16)
    l = (x - h.astype(np.float64) - m.astype(np.float64)).astype(bf16)
    return h, m, l


def _dist_rows(u, v):
    """u: (n,3) up points, v: (m,3) down points (fp32).
    Rows for dist^T[m,n] + eps = vr.T @ ur with K=25 bf16 rows, ordered
    large-terms-first so PSUM accumulation error stays small."""
    u = np.asarray(u, np.float64)
    v = np.asarray(v, np.float64)
    n, m = u.shape[0], v.shape[0]
    a2 = (u * u).sum(1)
    b2 = (v * v).sum(1)
    vs = -2.0 * v
    a2h, a2m, a2l = _split3(a2)
    b2h, b2m, b2l = _split3(b2)
    uh, um, ul = _split3(u)
    vh, vm, vl = _split3(vs)
    on = np.ones(n, bf16)
    om = np.ones(m, bf16)
    ur = np.stack([
        a2h, on, uh[:, 0], uh[:, 1], uh[:, 2],
        a2m, on, uh[:, 0], uh[:, 1], uh[:, 2], um[:, 0], um[:, 1], um[:, 2],
        a2l, on, uh[:, 0], uh[:, 1], uh[:, 2], um[:, 0], um[:, 1], um[:, 2],
        ul[:, 0], ul[:, 1], ul[:, 2], on,
    ])
    vr = np.stack([
        om, b2h, vh[:, 0], vh[:, 1], vh[:, 2],
        om, b2m, vm[:, 0], vm[:, 1], vm[:, 2], vh[:, 0], vh[:, 1], vh[:, 2],
        om, b2l, vl[:, 0], vl[:, 1], vl[:, 2], vm[:, 0], vm[:, 1], vm[:, 2],
        vh[:, 0], vh[:, 1], vh[:, 2], np.full(m, DIST_EPS, bf16),
    ])
    return ur, vr


def _act_direct(nc, func, out_ap, in_ap):
    """InstActivation with immediate bias/scale, bypassing the bass wrapper
    (which rejects Reciprocal wholesale; measured 1.2e-5 max rel err on HW
    over [1e-5, 100] - fine here, fp32r quantization dominates)."""
    eng = nc.scalar
    ins_l = [eng.lower_ap(in_ap)]
    for v in (0.0, 1.0, 0.0):  # bias, scale, alpha
        ins_l.append(mybir.ImmediateValue(dtype=dt.float32, value=v))
    eng.add_instruction(
        mybir.InstActivation(
            name=nc.get_next_instruction_name(),
            func=func,
            ins=ins_l,
            outs=[eng.lower_ap(out_ap)],
        )
    )


def _build_program(reps=1):
    nc = bacc.Bacc("TRN2", target_bir_lowering=False, debug=False)

    ur = nc.dram_tensor("ur", [KD, NN], dt.bfloat16, kind="ExternalInput")
    vr = nc.dram_tensor("vr", [KD, M], dt.bfloat16, kind="ExternalInput")
    fdr = nc.dram_tensor("fdr", [P, MT, D + 1], dt.float32r, kind="ExternalInput")
    fup = nc.dram_tensor("fup", [C, NN], dt.float32r, kind="ExternalInput")
    w1a = nc.dram_tensor("w1a", [C, C + D], dt.float32r, kind="ExternalInput")
    w1b = nc.dram_tensor("w1b", [D, C + D], dt.float32r, kind="ExternalInput")
    w2 = nc.dram_tensor("w2", [C + D, D], dt.float32r, kind="ExternalInput")
    bn1s = nc.dram_tensor("bn1s", [C + D, 1], dt.float32, kind="ExternalInput")
    bn1t = nc.dram_tensor("bn1t", [C + D, 1], dt.float32, kind="ExternalInput")
    bn2s = nc.dram_tensor("bn2s", [D, 1], dt.float32, kind="ExternalInput")
    bn2t = nc.dram_tensor("bn2t", [D, 1], dt.float32, kind="ExternalInput")
    outT = nc.dram_tensor("outT", [D, NN], dt.float32, kind="ExternalOutput")

    with tile.TileContext(nc) as tc:
        with (
            tc.tile_pool(name="const", bufs=1) as const,
            tc.tile_pool(name="rp", bufs=20) as rp_pool,
            tc.tile_pool(name="ft", bufs=3) as ft_pool,
            tc.tile_pool(name="it", bufs=3) as it_pool,
            tc.tile_pool(name="acs", bufs=3) as acs_pool,
            tc.tile_pool(name="h", bufs=3) as h_pool,
            tc.tile_pool(name="rs", bufs=3) as rs_pool,
            tc.tile_pool(name="rsb", bufs=3) as rsb_pool,
            tc.tile_pool(name="ot", bufs=3) as ot_pool,
            tc.tile_pool(name="pd", bufs=3, space="PSUM") as pd_pool,
            tc.tile_pool(name="pacc", bufs=2, space="PSUM") as pacc_pool,
            tc.tile_pool(name="py1", bufs=1, space="PSUM") as py1_pool,
            tc.tile_pool(name="py2", bufs=1, space="PSUM") as py2_pool,
        ):
            ur_s = const.tile([KD, NN], dt.bfloat16, tag="ur")
            vr_s = const.tile([KD, M], dt.bfloat16, tag="vr")
            fdr_s = const.tile([P, MT, D + 1], dt.float32r, tag="fdr")
            w1a_s = const.tile([C, C + D], dt.float32r, tag="w1a")
            w1b_s = const.tile([D, C + D], dt.float32r, tag="w1b")
            w2_s = const.tile([C + D, D], dt.float32r, tag="w2")
            bn1s_s = const.tile([C + D, 1], dt.float32, tag="bn1s")
            bn1t_s = const.tile([C + D, 1], dt.float32, tag="bn1t")
            bn2s_s = const.tile([D, 1], dt.float32, tag="bn2s")
            bn2t_s = const.tile([D, 1], dt.float32, tag="bn2t")
            nc.sync.dma_start(ur_s[:], ur.ap())
            nc.sync.dma_start(vr_s[:], vr.ap())
            nc.sync.dma_start(fdr_s[:], fdr.ap())
            nc.sync.dma_start(w1a_s[:], w1a.ap())
            nc.sync.dma_start(w1b_s[:], w1b.ap())
            nc.sync.dma_start(w2_s[:], w2.ap())
            nc.sync.dma_start(bn1s_s[:], bn1s.ap())
            nc.sync.dma_start(bn1t_s[:], bn1t.ap())
            nc.sync.dma_start(bn2s_s[:], bn2s.ap())
            nc.sync.dma_start(bn2t_s[:], bn2t.ap())

            for t in range(TT * reps):
                t = t % TT
                nsl = ts(t, NT)
                ft = ft_pool.tile([C, NT], dt.float32r, tag="ft")
                nc.sync.dma_start(ft[:], fup.ap()[:, nsl])

                acc = pacc_pool.tile([D + 1, NT], dt.float32, tag="acc")
                rps = []
                for mc in range(MT):
                    pd = pd_pool.tile([P, NT], dt.float32, tag="pd")
                    nc.tensor.matmul(pd[:], vr_s[:, ts(mc, P)], ur_s[:, nsl],
                                     start=True, stop=True)
                    rp = rp_pool.tile([P, NT], dt.float32r, tag="rp")
                    if mc < N_ACT_RECIP:
                        _act_direct(nc, AF.Reciprocal, rp[:], pd[:])
                    else:
                        nc.vector._custom_dve(RECIPROCAL_APPROX_FAST,
                                              out=rp[:], in0=pd[:], **RAF)
                    rps.append(rp)
                for mc in range(MT):
                    nc.tensor.matmul(acc[:], fdr_s[:, mc, :], rps[mc][:],
                                     start=(mc == 0), stop=(mc == MT - 1))

                # evict acc to SBUF; sum row -> partition 0 via DMA (engines
                # cannot move data across partitions)
                acs = acs_pool.tile([D + 1, NT], dt.float32, tag="acs")
                nc.scalar.copy(acs[:], acc[:])
                ssum = rs_pool.tile([1, NT], dt.float32, tag="ssum")
                nc.sync.dma_start(ssum[:], acs[D:D + 1, :])
                rs = rs_pool.tile([1, NT], dt.float32r, tag="rs")
                nc.vector._custom_dve(RECIPROCAL_APPROX_FAST,
                                      out=rs[:], in0=ssum[:], **RAF)
                rsb = rsb_pool.tile([D, NT], dt.float32r, tag="rsb")
                nc.gpsimd.partition_broadcast(rsb[:], rs[:])
                it = it_pool.tile([D, NT], dt.float32r, tag="it")
                nc.vector.tensor_tensor(it[:], acs[0:D, :], rsb[:],
                                        mybir.AluOpType.mult)

                y1 = py1_pool.tile([C + D, NT], dt.float32, tag="y1")
                nc.tensor.matmul(y1[:], w1a_s[:], ft[:], start=True, stop=False)
                nc.tensor.matmul(y1[:], w1b_s[:], it[:], start=False, stop=True)
                h = h_pool.tile([C + D, NT], dt.float32r, tag="h")
                nc.scalar.activation(h[:], y1[:], AF.Relu,
                                     bias=bn1t_s[:], scale=bn1s_s[:])
                y2 = py2_pool.tile([D, NT], dt.float32, tag="y2")
                nc.tensor.matmul(y2[:], w2_s[:], h[:], start=True, stop=True)
                ot = ot_pool.tile([D, NT], dt.float32, tag="ot")
                nc.vector.tensor_scalar(ot[:], y2[:], bn2s_s[:], bn2t_s[:],
                                        mybir.AluOpType.mult, mybir.AluOpType.add)
                nc.sync.dma_start(outT.ap()[:, nsl], ot[:])

    nc.compile()
    return nc


def _get_program(reps=1):
    if reps not in _compiled_cache:
        _compiled_cache[reps] = _build_program(reps)
    return _compiled_cache[reps]


def _prep_core_inputs(c, xyz_down, xyz_up, feat_down, feat_up,
                      W1, b1, bn1_gamma, bn1_beta, bn1_mean, bn1_var,
                      W2, b2, bn2_gamma, bn2_beta, bn2_mean, bn2_var):
    b = c // 2
    nsl = slice((c % 2) * NN, (c % 2) * NN + NN)
    ur, vr = _dist_rows(xyz_up[b, nsl], xyz_down[b])
    fd = np.concatenate([feat_down[b].astype(np.float32),
                         np.ones((M, 1), np.float32)], axis=1)       # (M, 65)
    fdr = np.ascontiguousarray(
        fd.reshape(MT, P, D + 1).transpose(1, 0, 2)).astype(np.float32)
    fup = np.ascontiguousarray(feat_up[b, nsl].astype(np.float32).T)  # (64, NN)
    s1 = (bn1_gamma / np.sqrt(bn1_var + BN_EPS)).astype(np.float64)
    t1 = (b1 - bn1_mean) * s1 + bn1_beta
    s2 = (bn2_gamma / np.sqrt(bn2_var + BN_EPS)).astype(np.float64)
    t2 = (b2 - bn2_mean) * s2 + bn2_beta
    w1t = W1.astype(np.float32).T
    return {
        "ur": ur, "vr": vr, "fdr": fdr, "fup": fup,
        "w1a": np.ascontiguousarray(w1t[0:C]),
        "w1b": np.ascontiguousarray(w1t[C:C + D]),
        "w2": np.ascontiguousarray(W2.astype(np.float32).T),
        "bn1s": np.asarray(s1, np.float32).reshape(-1, 1),
        "bn1t": np.asarray(t1, np.float32).reshape(-1, 1),
        "bn2s": np.asarray(s2, np.float32).reshape(-1, 1),
        "bn2t": np.asarray(t2, np.float32).reshape(-1, 1),
    }


def _run(inputs, reps=1, **kwargs):
    inputs = {k: np.asarray(v) for k, v in inputs.items()}
    in_maps = [_prep_core_inputs(c, **inputs) for c in range(NC)]
    nc = _get_program(reps)
    res = run_bass_kernel_spmd(nc, in_maps, core_ids=list(range(NC)), **kwargs)
    out = np.empty((B, N, D), np.float32)
    for c in range(NC):
        b = c // 2
        nsl = slice((c % 2) * NN, (c % 2) * NN + NN)
        out[b, nsl, :] = res.results[c]["outT"].T
    return out, res


def kernel(**inputs):
    out, _ = _run(inputs)
    return out


if __name__ == "__main__":
    rng = np.random.default_rng(0)
    demo = {
        "xyz_down": rng.standard_normal((B, M, 3), np.float32),
        "xyz_up": rng.standard_normal((B, N, 3), np.float32),
        "feat_down": rng.standard_normal((B, M, D), np.float32),
        "feat_up": rng.standard_normal((B, N, C), np.float32),
        "W1": (rng.standard_normal((C + D, C + D)) * 0.05).astype(np.float32),
        "b1": (rng.standard_normal(C + D) * 0.05).astype(np.float32),
        "bn1_gamma": rng.uniform(0.5, 1.5, C + D).astype(np.float32),
        "bn1_beta": (rng.standard_normal(C + D) * 0.05).astype(np.float32),
        "bn1_mean": (rng.standard_normal(C + D) * 0.05).astype(np.float32),
        "bn1_var": rng.uniform(0.5, 1.5, C + D).astype(np.float32),
        "W2": (rng.standard_normal((D, C + D)) * 0.05).astype(np.float32),
        "b2": (rng.standard_normal(D) * 0.05).astype(np.float32),
        "bn2_gamma": rng.uniform(0.5, 1.5, D).astype(np.float32),
        "bn2_beta": (rng.standard_normal(D) * 0.05).astype(np.float32),
        "bn2_mean": (rng.standard_normal(D) * 0.05).astype(np.float32),
        "bn2_var": rng.uniform(0.5, 1.5, D).astype(np.float32),
    }
    out = kernel(**demo)
    print("kernel ran, out", out.shape, out.dtype, np.abs(out).max())
